# revision 20
# baseline (speedup 1.0000x reference)
"""ActiveBoundaryLoss on 8 TRN2 NeuronCores (Bass/Tile), v2.

Sharding: core i handles image b=i//2, row half hf=i%2 (256 rows x 512 cols).
Partitions = columns (4 tiles x 128), free dims = [rows, class] with the
class dim padded 19->20 so bf16 rows stay 4B-aligned (2x DVE modes).

Math: with S(p) = sum_c exp(x(p,c)), lsm = x - lnS, sm = exp(x)/S:
  E_d(p) = KL(p, p+d) = sum_c sm(p)(x(p) - x(p+d)) - lnS(p) + lnS(p+d)
so the 9 per-pixel dot products run on (sm, x) directly in bf16 and no
log-softmax map is materialized.  CE = sum lnS - sum onehot*x.
Column (dy) shifts are SBUF-SBUF / DRAM partition-shift DMAs; row (dx)
shifts are free-dim offsets.  The eps AllReduce overlaps eps-independent
phase-D work (dist argmin, K stacks, LSE, dilation).
"""
import numpy as np
from contextlib import ExitStack
from ml_dtypes import bfloat16

import concourse.bass as bass
import concourse.bacc as bacc
import concourse.tile as tile
from concourse import mybir
from concourse.bass_utils import run_bass_kernel_spmd

ALU = mybir.AluOpType
ACTF = mybir.ActivationFunctionType
F32 = mybir.dt.float32
BF16 = mybir.dt.bfloat16
AX = mybir.AxisListType

B, C, H, W = 4, 19, 512, 512
CP = 20            # padded classes
OWN = 256          # rows per core
WIN = OWN + 4      # x window rows (halo 2 each side)
EW = OWN + 2       # E/klc window rows (halo 1 each side)
NCORES = 8
KEPS = 96
MAX_N = float(H * W * 0.01)
XPAD = -60.0       # class-pad logit: exp ~ 0, lsm finite
DIRS = [(1, 0), (-1, 0), (0, -1), (0, 1), (-1, 1), (1, 1), (-1, -1), (1, -1)]
NEG = [DIRS.index((-dx, -dy)) for (dx, dy) in DIRS]
LB_NEG = 0.2 / 8.0
LB_POS = 0.8
SSUM = LB_POS + 7.0 * LB_NEG  # 0.975
DEBUG = False

# D9 slab order: per-dy groups of 3 contiguous slabs (dir index; 8 = center)
ORD = [0, 1, 8, 2, 6, 7, 3, 4, 5]
GRP = [(0, 0), (3, -1), (6, 1)]              # (slab offset, dy)
ORDE = [0, 1, 2, 6, 7, 3, 4, 5]              # E8 slab j holds dir ORDE[j]
SLE = {d: j for j, d in enumerate(ORDE)}     # dir -> E8 slab
D9SL = {d: i for i, d in enumerate(ORD)}     # dir(+8=center) -> D9 slab


def _eps_list():
    t, e = [], np.float32(1e-5)
    for _ in range(KEPS):
        t.append(float(e))
        e = e * np.float32(1.2)
    return t


EPS_LIST = _eps_list()


def _bcast_part(ap, p=128):
    return bass.AP(tensor=ap.tensor, offset=ap.offset,
                   ap=[[0, p]] + list(ap.ap)[1:])


def tree_add20(nc, sc, src, out, M):
    """out (AP [128, M, 1] f32) = sum over the 20-wide inner axis of src
    (AP [128, M, 20] bf16, inner contiguous). sc = scratch tile dict.
    16+4 split keeps the wide levels 4B-aligned/bf16; the last levels run
    f32 so large partial sums don't get re-rounded to bf16."""
    t8, t4, t2, u2, t1, u1 = (sc[k] for k in ("t8", "t4", "t2", "u2", "t1", "u1"))
    nc.vector.tensor_tensor(out=t8[:, 0:M, :], in0=src[:, :, 0:8],
                            in1=src[:, :, 8:16], op=ALU.add)
    nc.vector.tensor_tensor(out=t4[:, 0:M, :], in0=t8[:, 0:M, 0:4],
                            in1=t8[:, 0:M, 4:8], op=ALU.add)
    nc.vector.tensor_tensor(out=t2[:, 0:M, :], in0=t4[:, 0:M, 0:2],
                            in1=t4[:, 0:M, 2:4], op=ALU.add)
    nc.vector.tensor_tensor(out=u2[:, 0:M, :], in0=src[:, :, 16:18],
                            in1=src[:, :, 18:20], op=ALU.add)
    nc.vector.tensor_tensor(out=t1[:, 0:M, :], in0=t2[:, 0:M, 0:1],
                            in1=t2[:, 0:M, 1:2], op=ALU.add)
    nc.vector.tensor_tensor(out=u1[:, 0:M, :], in0=u2[:, 0:M, 0:1],
                            in1=u2[:, 0:M, 1:2], op=ALU.add)
    nc.vector.tensor_tensor(out=out, in0=t1[:, 0:M, :], in1=u1[:, 0:M, :],
                            op=ALU.add)


def build_nc(sim=False):
    nc = bacc.Bacc("TRN2", target_bir_lowering=False, debug=False,
                   num_devices=1 if sim else NCORES)
    xw = nc.declare_dram_parameter("xw", [W, WIN, CP], BF16, isOutput=False)
    ohe = nc.declare_dram_parameter("ohe", [W, OWN, CP], BF16, isOutput=False)
    dw = nc.declare_dram_parameter("dw", [W + 2, EW], F32, isOutput=False)
    msk = nc.declare_dram_parameter("msk", [1, 2], F32, isOutput=False)
    etab = nc.declare_dram_parameter("etab", [1, 128], F32, isOutput=False)
    outp = nc.declare_dram_parameter("res", [1, 1], F32, isOutput=True)
    if DEBUG:
        dbgG = nc.declare_dram_parameter("dbgG", [1, 8], F32, isOutput=True)
        dbgC = nc.declare_dram_parameter("dbgC", [1, 128], F32, isOutput=True)
        dbgR = nc.declare_dram_parameter("dbgR", [1, 8], F32, isOutput=True)

    cnt_in = nc.dram_tensor("cnt_in", [1, 128], F32)
    cnt_out = nc.dram_tensor("cnt_out", [1, 128], F32, addr_space="Shared")
    fin_in = nc.dram_tensor("fin_in", [1, 8], F32)
    fin_out = nc.dram_tensor("fin_out", [1, 8], F32, addr_space="Shared")
    groups = [list(range(NCORES))]

    with tile.TileContext(nc) as tc, ExitStack() as ctx:
        keep = ctx.enter_context(tc.tile_pool(name="keep", bufs=1))
        big = ctx.enter_context(tc.tile_pool(name="big", bufs=1))
        tmp = ctx.enter_context(tc.tile_pool(name="tmp", bufs=1))
        med = ctx.enter_context(tc.tile_pool(name="med", bufs=1))
        psum = ctx.enter_context(tc.tile_pool(name="psum", bufs=1, space="PSUM"))
        dram = ctx.enter_context(tc.tile_pool(name="dram", bufs=1, space="DRAM"))

        # ---- persistent tiles ------------------------------------------
        lnS = [keep.tile([128, WIN], F32, tag=f"lnS{t}", name=f"lnS{t}")
               for t in range(4)]
        rS4 = [keep.tile([128, WIN], BF16, tag=f"rS{t}", name=f"rS{t}")
               for t in range(4)]
        E8 = [keep.tile([128, 8, EW], BF16, tag=f"E8{t}", name=f"E8{t}")
              for t in range(4)]
        klc4 = keep.tile([128, 4, EW], F32, tag="klc4")
        counts = keep.tile([128, KEPS], F32, tag="counts")
        stats = keep.tile([128, 5, 4], F32, tag="stats")
        nc.vector.memset(stats, 0.0)
        lsce4 = keep.tile([128, 4, OWN], BF16, tag="lsce4")
        M4 = keep.tile([128, 4, OWN], F32, tag="M4")
        nble4 = keep.tile([128, 4, OWN], BF16, tag="nble4")
        w4 = keep.tile([128, 4, OWN], BF16, tag="w4")
        mskb = keep.tile([128, 2], F32, tag="mskb")
        nc.sync.dma_start(out=mskb, in_=_bcast_part(msk[:]))
        etab_sb = keep.tile([1, 128], F32, tag="etab_sb")
        nc.sync.dma_start(out=etab_sb, in_=etab[:])
        ones = keep.tile([128, 1], F32, tag="ones")
        nc.vector.memset(ones, 1.0)
        cidx = keep.tile([128, 8], BF16, tag="cidx")
        for d in range(8):
            nc.vector.memset(cidx[:, d:d + 1], float(d))
        zrow = keep.tile([1, EW], F32, tag="zrow")
        nc.vector.memset(zrow, 0.0)
        zrowb = keep.tile([1, EW], BF16, tag="zrowb")
        nc.vector.memset(zrowb, 0.0)
        eps_dr = dram.tile([1, 1], F32, tag="eps_dr")

        # big pool: [128, WIN, CP] bf16 buffers, tags reused across phases
        def bigt(tag):
            return big.tile([128, WIN, CP], BF16, tag=tag, name=tag)

        # tree scratch (3-dir batched trees; S-tree uses M=WIN slice)
        MX = 3 * EW
        sc = {
            "t8": tmp.tile([128, MX, 8], BF16, tag="sc_t8", name="sc_t8"),
            "t4": tmp.tile([128, MX, 4], BF16, tag="sc_t4", name="sc_t4"),
            "t2": tmp.tile([128, MX, 2], F32, tag="sc_t2", name="sc_t2"),
            "u2": tmp.tile([128, MX, 2], BF16, tag="sc_u2", name="sc_u2"),
            "t1": tmp.tile([128, MX, 1], F32, tag="sc_t1", name="sc_t1"),
            "u1": tmp.tile([128, MX, 1], F32, tag="sc_u1", name="sc_u1"),
        }
        prod = tmp.tile([128, MX, CP], BF16, tag="prod")

        # ================= Phase A: exp / S / lnS / 1/S / CE ============
        for t in range(4):
            p0 = t * 128
            xc = bigt("xc")
            nc.sync.dma_start(out=xc, in_=xw[p0:p0 + 128])
            ex = bigt("ex")
            nc.scalar.activation(out=ex, in_=xc, func=ACTF.Exp)
            S = med.tile([128, WIN, 1], F32, tag="S")
            tree_add20(nc, sc, ex[:, :, :], S[:], WIN)
            rSf = med.tile([128, WIN, 1], F32, tag="rSf")
            nc.vector.reciprocal_approx_fast(out=rSf, in_=S)
            nc.scalar.activation(out=rS4[t], in_=rSf[:, :, 0], func=ACTF.Copy)
            # lnS (scalar engine); own rows also accumulate sum(lnS)
            nc.scalar.activation(out=lnS[t][:, 0:2], in_=S[:, 0:2, 0],
                                 func=ACTF.Ln)
            nc.scalar.activation(out=lnS[t][:, 2:2 + OWN],
                                 in_=S[:, 2:2 + OWN, 0], func=ACTF.Ln,
                                 accum_out=stats[:, 3, t:t + 1])
            nc.scalar.activation(out=lnS[t][:, 2 + OWN:WIN],
                                 in_=S[:, 2 + OWN:WIN, 0], func=ACTF.Ln)
            # CE data term: sum(onehot * x) over own rows
            oht = bigt("xl")
            nc.sync.dma_start(out=oht[:, 0:OWN, :], in_=ohe[p0:p0 + 128])
            junk = bigt("xr")
            nc.vector.scalar_tensor_tensor(
                out=junk[:, 0:OWN, :], in0=oht[:, 0:OWN, :], scalar=1.0,
                in1=xc[:, 2:2 + OWN, :], op0=ALU.mult, op1=ALU.mult,
                accum_out=stats[:, 4, t:t + 1])

        # ================= Phase B: 9 dots -> E8 / klc ==================
        for t in range(4):
            p0 = t * 128
            xc = bigt("xc")
            nc.sync.dma_start(out=xc, in_=xw[p0:p0 + 128])
            xL = bigt("xl")
            if t > 0:
                nc.sync.dma_start(out=xL, in_=xw[p0 - 1:p0 + 127])
            else:
                nc.sync.dma_start(out=xL[1:128], in_=xw[0:127])
                nc.sync.dma_start(out=xL[0:1], in_=xw[0:1])
            xR = bigt("xr")
            if t < 3:
                nc.sync.dma_start(out=xR, in_=xw[p0 + 1:p0 + 129])
            else:
                nc.sync.dma_start(out=xR[0:127], in_=xw[p0 + 1:p0 + 128])
                nc.sync.dma_start(out=xR[127:128], in_=xw[W - 1:W])
            lnL = med.tile([128, WIN], F32, tag="lnL")
            nc.sync.dma_start(out=lnL[1:128], in_=lnS[t][0:127])
            nc.sync.dma_start(out=lnL[0:1],
                              in_=(lnS[t - 1][127:128] if t > 0 else lnS[0][0:1]))
            lnR = med.tile([128, WIN], F32, tag="lnR")
            nc.sync.dma_start(out=lnR[0:127], in_=lnS[t][1:128])
            nc.sync.dma_start(out=lnR[127:128],
                              in_=(lnS[t + 1][0:1] if t < 3 else lnS[3][127:128]))
            # sm = exp(x) * (1/S)
            ex = bigt("ex")
            nc.scalar.activation(out=ex, in_=xc, func=ACTF.Exp)
            sm = bigt("sm")
            nc.vector.tensor_tensor(
                out=sm, in0=ex,
                in1=rS4[t][:].unsqueeze(2).broadcast_to([128, WIN, CP]),
                op=ALU.mult)

            D9 = med.tile([128, 9 * EW], F32, tag="D9")
            smc = sm[:, 1:1 + EW, :]
            lnX = {0: lnS[t], -1: lnL, 1: lnR}
            for (goff, dy) in GRP:
                xs = {0: xc, -1: xL, 1: xR}[dy]
                for j in range(3):
                    d = ORD[goff + j]
                    dx = 0 if d == 8 else DIRS[d][0]
                    nc.vector.tensor_tensor(
                        out=prod[:, j * EW:(j + 1) * EW, :], in0=smc,
                        in1=xs[:, 1 + dx:1 + dx + EW, :], op=ALU.mult)
                tree_add20(nc, sc, prod[:, 0:3 * EW, :],
                           D9[:, goff * EW:(goff + 3) * EW].unsqueeze(2),
                           3 * EW)

            # E8 = (A - D_d) + (lnS_d - lnS); A = D9 slab 2 (center)
            Abc2 = D9[:, 2 * EW:3 * EW].unsqueeze(1).broadcast_to([128, 2, EW])
            Abc6 = D9[:, 2 * EW:3 * EW].unsqueeze(1).broadcast_to([128, 6, EW])
            ad8 = med.tile([128, 8, EW], BF16, tag="b8c")
            nc.vector.scalar_tensor_tensor(
                out=ad8[:, 0:2, :],
                in0=D9[:, 0:2 * EW].rearrange("p (d e) -> p d e", e=EW),
                scalar=-1.0, in1=Abc2, op0=ALU.mult, op1=ALU.add)
            nc.vector.scalar_tensor_tensor(
                out=ad8[:, 2:8, :],
                in0=D9[:, 3 * EW:9 * EW].rearrange("p (d e) -> p d e", e=EW),
                scalar=-1.0, in1=Abc6, op0=ALU.mult, op1=ALU.add)
            Ld = med.tile([128, 8, EW], BF16, tag="b8d")
            lnSc = lnS[t][:, 1:1 + EW]
            for d in range(8):
                dx, dy = DIRS[d]
                nc.vector.tensor_tensor(
                    out=Ld[:, SLE[d], :],
                    in0=lnX[dy][:, 1 + dx:1 + dx + EW], in1=lnSc,
                    op=ALU.subtract)
            nc.vector.tensor_tensor(out=E8[t], in0=ad8, in1=Ld, op=ALU.add)

            # klc = E_(1,0) + E_(0,1) in f32 (threshold path needs the
            # precision; the bf16 E8 copy only feeds the lsce path)
            klc = klc4[:, t, :]
            e0f = med.tile([128, EW], F32, tag="e0f")
            nc.vector.scalar_tensor_tensor(
                out=e0f, in0=D9[:, 0:EW], scalar=-1.0,
                in1=D9[:, 2 * EW:3 * EW], op0=ALU.mult, op1=ALU.add)
            nc.vector.tensor_tensor(out=e0f, in0=e0f,
                                    in1=lnS[t][:, 2:2 + EW], op=ALU.add)
            e3f = med.tile([128, EW], F32, tag="e3f")
            nc.vector.scalar_tensor_tensor(
                out=e3f, in0=D9[:, 6 * EW:7 * EW], scalar=-1.0,
                in1=D9[:, 2 * EW:3 * EW], op0=ALU.mult, op1=ALU.add)
            nc.vector.tensor_tensor(out=e3f, in0=e3f,
                                    in1=lnR[:, 1:1 + EW], op=ALU.add)
            nc.vector.tensor_tensor(out=e0f, in0=e0f, in1=e3f, op=ALU.add)
            nc.vector.scalar_tensor_tensor(
                out=klc, in0=lnS[t][:, 1:1 + EW], scalar=-2.0,
                in1=e0f, op0=ALU.mult, op1=ALU.add)
            nc.vector.tensor_tensor(out=klc[:, 0:1], in0=klc[:, 0:1],
                                    in1=mskb[:, 0:1], op=ALU.mult)
            nc.vector.tensor_tensor(out=klc[:, EW - 1:EW], in0=klc[:, EW - 1:EW],
                                    in1=mskb[:, 1:2], op=ALU.mult)

        # ================= counts + AllReduce (overlapped) ==============
        for k in range(KEPS):
            nc.vector.tensor_scalar(out=M4, in0=klc4[:, :, 1:1 + OWN],
                                    scalar1=EPS_LIST[k], scalar2=0.0,
                                    op0=ALU.is_gt, op1=ALU.add,
                                    accum_out=counts[:, k:k + 1])
        cred = psum.tile([1, KEPS], F32, tag="cred")
        nc.tensor.matmul(cred, ones, counts, start=True, stop=True)
        cred_sb = keep.tile([1, KEPS], F32, tag="cred_sb")
        nc.vector.tensor_copy(out=cred_sb, in_=cred)
        nc.sync.dma_start(out=cnt_in[:, 0:KEPS], in_=cred_sb)
        if sim:
            nc.sync.dma_start(out=cnt_out[:, 0:KEPS], in_=cnt_in[:, 0:KEPS])
        else:
            nc.gpsimd.collective_compute(
                "AllReduce", ALU.add, replica_groups=groups,
                ins=[cnt_in[:, 0:KEPS]], outs=[cnt_out[:, 0:KEPS]])

        # ========== Phase D (eps-independent), overlaps the AllReduce ===
        for t in range(4):
            p0 = t * 128
            # dist9 stack from DRAM (slab order: DIRS + center at 8)
            D9d = med.tile([128, 9 * EW], F32, tag="D9")

            def dsl(di):
                return D9d[:, di * EW:di * EW + OWN]

            for di in range(9):
                dx, dy = (0, 0) if di == 8 else DIRS[di]
                nc.sync.dma_start(
                    out=dsl(di),
                    in_=dw[p0 + 1 + dy:p0 + 129 + dy][:, 1 + dx:1 + dx + OWN])
            f8b = med.tile([128, 4, EW], F32, tag="f8b")
            m4a, m2a = f8b[:, 0:4, 0:OWN], f8b[:, 0:2, 0:OWN]
            b8a = med.tile([128, 8, OWN], BF16, tag="b8a")  # pen/eqs/dsub
            b8b = med.tile([128, 8, OWN], BF16, tag="b8b")  # kp/dexp
            b4 = med.tile([128, 4, OWN], BF16, tag="b4")
            b2 = med.tile([128, 2, OWN], BF16, tag="b2")
            d8v = D9d[:, 0:8 * EW].rearrange(
                "p (d e) -> p d e", e=EW)[:, :, 0:OWN]
            nc.vector.tensor_tensor(out=m4a, in0=d8v[:, 0:4, :],
                                    in1=d8v[:, 4:8, :], op=ALU.min)
            nc.vector.tensor_tensor(out=m2a, in0=m4a[:, 0:2, :],
                                    in1=m4a[:, 2:4, :], op=ALU.min)
            min8 = med.tile([128, OWN], F32, tag="min8")
            nc.vector.tensor_tensor(out=min8, in0=m2a[:, 0, :],
                                    in1=m2a[:, 1, :], op=ALU.min)
            nc.vector.tensor_tensor(out=nble4[:, t, :], in0=min8,
                                    in1=dsl(8), op=ALU.is_le)
            nc.vector.tensor_scalar(out=w4[:, t, :], in0=dsl(8),
                                    scalar1=20.0, scalar2=0.05,
                                    op0=ALU.min, op1=ALU.mult)
            # first-argmin index among the 8 dirs (small ints: exact bf16)
            nc.vector.tensor_tensor(
                out=b8a, in0=d8v,
                in1=min8[:].unsqueeze(1).broadcast_to([128, 8, OWN]),
                op=ALU.is_equal)
            nc.vector.scalar_tensor_tensor(
                out=b8a, in0=b8a, scalar=-100.0, op0=ALU.mult,
                in1=cidx[:].unsqueeze(2).broadcast_to([128, 8, OWN]),
                op1=ALU.add)
            nc.vector.tensor_tensor(out=b4, in0=b8a[:, 0:4, :],
                                    in1=b8a[:, 4:8, :], op=ALU.min)
            nc.vector.tensor_tensor(out=b2, in0=b4[:, 0:2, :],
                                    in1=b4[:, 2:4, :], op=ALU.min)
            dgt = med.tile([128, OWN], BF16, tag="dgt")
            nc.vector.tensor_tensor(out=dgt, in0=b2[:, 0, :],
                                    in1=b2[:, 1, :], op=ALU.min)
            nc.vector.tensor_scalar(out=dgt, in0=dgt, scalar1=100.0,
                                    scalar2=None, op0=ALU.add)

            # K8 stack: K_d = E_{NEG[d]} shifted by d
            K8 = med.tile([128, 8, OWN], BF16, tag="K8")
            for di in range(8):
                dx, dy = DIRS[di]
                sl = SLE[NEG[di]]
                if dy == 0:
                    nc.scalar.activation(
                        out=K8[:, di, :],
                        in_=E8[t][:, sl, 1 + dx:1 + dx + OWN], func=ACTF.Copy)
                elif dy == -1:
                    nc.sync.dma_start(
                        out=K8[1:128, di, :],
                        in_=E8[t][0:127, sl, 1 + dx:1 + dx + OWN])
                    if t > 0:
                        nc.sync.dma_start(
                            out=K8[0:1, di, :],
                            in_=E8[t - 1][127:128, sl, 1 + dx:1 + dx + OWN])
                    elif dx != 0:
                        gsl = SLE[DIRS.index((-dx, 0))]
                        nc.sync.dma_start(
                            out=K8[0:1, di, :],
                            in_=E8[0][0:1, gsl, 1 + dx:1 + dx + OWN])
                    else:
                        nc.sync.dma_start(out=K8[0:1, di, :],
                                          in_=zrowb[:, 0:OWN])
                else:
                    nc.sync.dma_start(
                        out=K8[0:127, di, :],
                        in_=E8[t][1:128, sl, 1 + dx:1 + dx + OWN])
                    if t < 3:
                        nc.sync.dma_start(
                            out=K8[127:128, di, :],
                            in_=E8[t + 1][0:1, sl, 1 + dx:1 + dx + OWN])
                    elif dx != 0:
                        gsl = SLE[DIRS.index((-dx, 0))]
                        nc.sync.dma_start(
                            out=K8[127:128, di, :],
                            in_=E8[3][127:128, gsl, 1 + dx:1 + dx + OWN])
                    else:
                        nc.sync.dma_start(out=K8[127:128, di, :],
                                          in_=zrowb[:, 0:OWN])

            # ksel = K8[dgt] via equality select
            nc.vector.tensor_tensor(
                out=b8a, in0=cidx[:].unsqueeze(2).broadcast_to([128, 8, OWN]),
                in1=dgt[:].unsqueeze(1).broadcast_to([128, 8, OWN]),
                op=ALU.is_equal)
            nc.vector.tensor_tensor(out=b8b, in0=b8a, in1=K8, op=ALU.mult)
            nc.vector.tensor_tensor(out=b4, in0=b8b[:, 0:4, :],
                                    in1=b8b[:, 4:8, :], op=ALU.add)
            nc.vector.tensor_tensor(out=b2, in0=b4[:, 0:2, :],
                                    in1=b4[:, 2:4, :], op=ALU.add)
            ksel = med.tile([128, OWN], BF16, tag="ksel")
            nc.vector.tensor_tensor(out=ksel, in0=b2[:, 0, :],
                                    in1=b2[:, 1, :], op=ALU.add)

            # LSE + plain sum over the 8 K maps
            nc.vector.tensor_tensor(out=b4, in0=K8[:, 0:4, :],
                                    in1=K8[:, 4:8, :], op=ALU.max)
            nc.vector.tensor_tensor(out=b2, in0=b4[:, 0:2, :],
                                    in1=b4[:, 2:4, :], op=ALU.max)
            m8 = med.tile([128, OWN], BF16, tag="m8")
            nc.vector.tensor_tensor(out=m8, in0=b2[:, 0, :],
                                    in1=b2[:, 1, :], op=ALU.max)
            nc.vector.tensor_tensor(
                out=b8a, in0=K8,
                in1=m8[:].unsqueeze(1).broadcast_to([128, 8, OWN]),
                op=ALU.subtract)
            nc.scalar.activation(out=b8b, in_=b8a, func=ACTF.Exp)
            nc.vector.tensor_tensor(out=b4, in0=b8b[:, 0:4, :],
                                    in1=b8b[:, 4:8, :], op=ALU.add)
            nc.vector.tensor_tensor(out=b2, in0=b4[:, 0:2, :],
                                    in1=b4[:, 2:4, :], op=ALU.add)
            esum = med.tile([128, OWN], BF16, tag="esum")
            nc.vector.tensor_tensor(out=esum, in0=b2[:, 0, :],
                                    in1=b2[:, 1, :], op=ALU.add)
            lnE = med.tile([128, OWN], BF16, tag="lnE")
            nc.scalar.activation(out=lnE, in_=esum, func=ACTF.Ln)
            lse = med.tile([128, OWN], BF16, tag="lse")
            nc.vector.tensor_tensor(out=lse, in0=m8, in1=lnE, op=ALU.add)
            nc.vector.tensor_tensor(out=b4, in0=K8[:, 0:4, :],
                                    in1=K8[:, 4:8, :], op=ALU.add)
            nc.vector.tensor_tensor(out=b2, in0=b4[:, 0:2, :],
                                    in1=b4[:, 2:4, :], op=ALU.add)
            s8 = med.tile([128, OWN], BF16, tag="s8")
            nc.vector.tensor_tensor(out=s8, in0=b2[:, 0, :],
                                    in1=b2[:, 1, :], op=ALU.add)
            # lsce = SSUM*lse - LB_NEG*s8 - (LB_POS-LB_NEG)*ksel
            a1 = med.tile([128, OWN], BF16, tag="a1")
            nc.vector.tensor_scalar(out=a1, in0=s8, scalar1=-LB_NEG,
                                    scalar2=None, op0=ALU.mult)
            b1 = med.tile([128, OWN], BF16, tag="b1")
            nc.vector.scalar_tensor_tensor(out=b1, in0=lse, scalar=SSUM,
                                           op0=ALU.mult, in1=a1, op1=ALU.add)
            nc.vector.scalar_tensor_tensor(out=lsce4[:, t, :], in0=ksel,
                                           scalar=-(LB_POS - LB_NEG),
                                           op0=ALU.mult, in1=b1, op1=ALU.add)

            # dilation M = 3x3 max of klc
            kC = klc4[:, t, :]
            kL = med.tile([128, EW], F32, tag="kL")
            if t > 0:
                nc.sync.dma_start(out=kL[0:1], in_=klc4[127:128, t - 1, :])
            else:
                nc.vector.memset(kL[0:1], 0.0)
            nc.sync.dma_start(out=kL[1:128], in_=klc4[0:127, t, :])
            kR = med.tile([128, EW], F32, tag="kR")
            if t < 3:
                nc.sync.dma_start(out=kR[127:128], in_=klc4[0:1, t + 1, :])
            else:
                nc.sync.dma_start(out=kR[127:128], in_=zrow)
            nc.sync.dma_start(out=kR[0:127], in_=klc4[1:128, t, :])
            M = M4[:, t, :]
            nc.vector.tensor_tensor(out=M, in0=kL[:, 0:OWN],
                                    in1=kL[:, 1:1 + OWN], op=ALU.max)
            nc.vector.tensor_tensor(out=M, in0=M, in1=kL[:, 2:2 + OWN],
                                    op=ALU.max)
            for src in (kC, kR):
                for rs in range(3):
                    nc.vector.tensor_tensor(out=M, in0=M,
                                            in1=src[:, rs:rs + OWN],
                                            op=ALU.max)

        # ================= eps + masked sums ============================
        tot = keep.tile([1, KEPS], F32, tag="tot")
        nc.sync.dma_start(out=tot, in_=cnt_out[:, 0:KEPS])
        maskT = keep.tile([1, KEPS], F32, tag="maskT")
        nc.vector.tensor_scalar(out=maskT, in0=tot, scalar1=MAX_N, scalar2=None,
                                op0=ALU.is_le)
        penal = keep.tile([1, KEPS], F32, tag="penal")
        nc.vector.tensor_scalar(out=penal, in0=maskT, scalar1=-1e30,
                                scalar2=1e30, op0=ALU.mult, op1=ALU.add)
        maskedT = keep.tile([1, KEPS], F32, tag="maskedT")
        nc.vector.tensor_tensor(out=maskedT, in0=etab_sb[:, 0:KEPS], in1=penal,
                                op=ALU.add)
        eps1 = keep.tile([1, 1], F32, tag="eps1")
        nc.vector.tensor_reduce(out=eps1, in_=maskedT, axis=AX.X, op=ALU.min)
        nc.sync.dma_start(out=eps_dr[:], in_=eps1)
        epsb = keep.tile([128, 1], F32, tag="epsb")
        nc.sync.dma_start(out=epsb, in_=_bcast_part(eps_dr[:]))

        for t in range(4):
            pb = med.tile([128, OWN], BF16, tag="pb")
            nc.vector.tensor_scalar(out=pb, in0=M4[:, t, :], scalar1=epsb,
                                    scalar2=None, op0=ALU.is_gt)
            vm = med.tile([128, OWN], BF16, tag="vm")
            nc.vector.tensor_tensor(out=vm, in0=pb, in1=nble4[:, t, :],
                                    op=ALU.mult)
            junkD = med.tile([128, OWN], BF16, tag="junkD")
            nc.vector.scalar_tensor_tensor(out=junkD, in0=lsce4[:, t, :],
                                           scalar=1.0, op0=ALU.mult, in1=vm,
                                           op1=ALU.mult,
                                           accum_out=stats[:, 0, t:t + 1])
            nc.vector.scalar_tensor_tensor(out=junkD, in0=w4[:, t, :],
                                           scalar=1.0, op0=ALU.mult, in1=vm,
                                           op1=ALU.mult,
                                           accum_out=stats[:, 1, t:t + 1])
            nc.vector.tensor_scalar(out=junkD, in0=pb, scalar1=1.0, scalar2=0.0,
                                    op0=ALU.mult, op1=ALU.add,
                                    accum_out=stats[:, 2, t:t + 1])

        # ================= final reduce + AllReduce + output ============
        red5 = keep.tile([128, 5], F32, tag="red5")
        nc.vector.tensor_reduce(out=red5, in_=stats, axis=AX.X, op=ALU.add)
        redr = psum.tile([1, 5], F32, tag="redr")
        nc.tensor.matmul(redr, ones, red5, start=True, stop=True)
        redr_sb = keep.tile([1, 5], F32, tag="redr_sb")
        nc.vector.tensor_copy(out=redr_sb, in_=redr)
        nc.sync.dma_start(out=fin_in[:, 0:5], in_=redr_sb)
        if sim:
            nc.sync.dma_start(out=fin_out[:, 0:5], in_=fin_in[:, 0:5])
        else:
            nc.gpsimd.collective_compute(
                "AllReduce", ALU.add, replica_groups=groups,
                ins=[fin_in[:, 0:5]], outs=[fin_out[:, 0:5]])
        G = keep.tile([1, 5], F32, tag="G")
        nc.sync.dma_start(out=G, in_=fin_out[:, 0:5])
        if DEBUG:
            nc.sync.dma_start(out=dbgG[:, 0:5], in_=G)
            nc.sync.dma_start(out=dbgG[:, 5:6], in_=eps1)
            nc.sync.dma_start(out=dbgC[:, 0:KEPS], in_=tot)
            nc.sync.dma_start(out=dbgR[:, 0:5], in_=redr_sb)
        gate = keep.tile([1, 1], F32, tag="gate")
        nc.vector.tensor_scalar(out=gate, in0=G[:, 2:3], scalar1=1.0,
                                scalar2=None, op0=ALU.is_gt)
        bl = keep.tile([1, 1], F32, tag="bl")
        nc.vector.tensor_tensor(out=bl, in0=G[:, 0:1], in1=G[:, 1:2],
                                op=ALU.mult)
        nc.vector.tensor_tensor(out=bl, in0=bl, in1=gate, op=ALU.mult)
        tl = keep.tile([1, 1], F32, tag="tl")
        nc.vector.tensor_tensor(out=tl, in0=G[:, 3:4], in1=G[:, 4:5],
                                op=ALU.subtract)
        res = keep.tile([1, 1], F32, tag="res")
        nc.vector.scalar_tensor_tensor(out=res, in0=bl, scalar=0.1,
                                       in1=tl, op0=ALU.mult, op1=ALU.add)
        nc.sync.dma_start(out=outp[:], in_=res)

    nc.compile()
    return nc


_NC = None


def _get_nc():
    global _NC
    if _NC is None:
        _NC = build_nc()
    return _NC


def kernel_in_maps(slices, dist_maps, targets):
    slices = np.asarray(slices, np.float32)
    dist_maps = np.asarray(dist_maps, np.float32)
    targets = np.asarray(targets)
    etab = np.zeros((1, 128), np.float32)
    etab[0, :KEPS] = EPS_LIST
    etab[0, KEPS:] = EPS_LIST[-1]
    in_maps = []
    for core in range(NCORES):
        b, hf = core // 2, core % 2
        r0 = hf * OWN
        rows = np.clip(np.arange(r0 - 2, r0 + OWN + 2), 0, H - 1)
        xwin = np.full((W, WIN, CP), XPAD, np.float32)
        xwin[:, :, :C] = np.transpose(slices[b][:, rows, :], (2, 1, 0))
        xwv = xwin.astype(bfloat16)
        tgt = targets[b, 0, r0:r0 + OWN]                     # [OWN, W]
        oh = np.zeros((W, OWN, CP), np.float32)
        ww, rr = np.meshgrid(np.arange(W), np.arange(OWN), indexing='ij')
        cc = np.clip(tgt.T, 0, C - 1)                        # [W, OWN]
        oh[ww, rr, cc] = np.where(tgt.T == 255, 0.0, 1.0)
        ohv = oh.astype(bfloat16)
        ridx = np.arange(r0 - 1, r0 + OWN + 1)
        inb = ((ridx >= 0) & (ridx < H))[:, None]
        dwin = np.where(inb, dist_maps[b, 0][np.clip(ridx, 0, H - 1)],
                        np.float32(1e5))                      # [EW, W]
        dwin = np.pad(dwin, ((0, 0), (1, 1)),
                      constant_values=np.float32(1e5))        # [EW, W+2]
        dwv = np.ascontiguousarray(dwin.T)                    # [W+2, EW]
        mskv = np.array([[1.0 if r0 > 0 else 0.0,
                          1.0 if r0 + OWN < H else 0.0]], np.float32)
        in_maps.append({"xw": xwv, "ohe": ohv, "dw": dwv, "msk": mskv,
                        "etab": etab})
    return in_maps


def kernel(slices, dist_maps, targets):
    in_maps = kernel_in_maps(slices, dist_maps, targets)
    nc = _get_nc()
    res = run_bass_kernel_spmd(nc, in_maps, list(range(NCORES)))
    out = np.asarray(res.results[0]["res"], np.float32)
    return out.reshape(())


# revision 23
# speedup vs baseline: 1.1405x; 1.1405x over previous
"""ActiveBoundaryLoss on 8 TRN2 NeuronCores (Bass/Tile), v2.

Sharding: core i handles image b=i//2, row half hf=i%2 (256 rows x 512 cols).
Partitions = columns (4 tiles x 128), free dims = [rows, class] with the
class dim padded 19->20 so bf16 rows stay 4B-aligned (2x DVE modes).

Math: with S(p) = sum_c exp(x(p,c)), lsm = x - lnS, sm = exp(x)/S:
  E_d(p) = KL(p, p+d) = sum_c sm(p)(x(p) - x(p+d)) - lnS(p) + lnS(p+d)
so the 9 per-pixel dot products run on (sm, x) directly in bf16 and no
log-softmax map is materialized.  CE = sum lnS - sum onehot*x.
Column (dy) shifts are SBUF-SBUF / DRAM partition-shift DMAs; row (dx)
shifts are free-dim offsets.  The eps AllReduce overlaps eps-independent
phase-D work (dist argmin, K stacks, LSE, dilation).
"""
import numpy as np
from contextlib import ExitStack
from ml_dtypes import bfloat16

import concourse.bass as bass
import concourse.bacc as bacc
import concourse.tile as tile
from concourse import mybir
from concourse.bass_utils import run_bass_kernel_spmd

ALU = mybir.AluOpType
ACTF = mybir.ActivationFunctionType
F32 = mybir.dt.float32
BF16 = mybir.dt.bfloat16
AX = mybir.AxisListType

B, C, H, W = 4, 19, 512, 512
CP = 20            # padded classes
OWN = 256          # rows per core
WIN = OWN + 4      # x window rows (halo 2 each side)
EW = OWN + 2       # E/klc window rows (halo 1 each side)
NCORES = 8
KEPS = 96
MAX_N = float(H * W * 0.01)
XPAD = -60.0       # class-pad logit: exp ~ 0, lsm finite
DIRS = [(1, 0), (-1, 0), (0, -1), (0, 1), (-1, 1), (1, 1), (-1, -1), (1, -1)]
NEG = [DIRS.index((-dx, -dy)) for (dx, dy) in DIRS]
LB_NEG = 0.2 / 8.0
LB_POS = 0.8
SSUM = LB_POS + 7.0 * LB_NEG  # 0.975
DEBUG = False

# D9 slab order: per-dy groups of 3 contiguous slabs (dir index; 8 = center)
ORD = [0, 1, 8, 2, 6, 7, 3, 4, 5]
GRP = [(0, 0), (3, -1), (6, 1)]              # (slab offset, dy)
ORDE = [0, 1, 2, 6, 7, 3, 4, 5]              # E8 slab j holds dir ORDE[j]
SLE = {d: j for j, d in enumerate(ORDE)}     # dir -> E8 slab
D9SL = {d: i for i, d in enumerate(ORD)}     # dir(+8=center) -> D9 slab


def _eps_list():
    t, e = [], np.float32(1e-5)
    for _ in range(KEPS):
        t.append(float(e))
        e = e * np.float32(1.2)
    return t


EPS_LIST = _eps_list()


def _bcast_part(ap, p=128):
    return bass.AP(tensor=ap.tensor, offset=ap.offset,
                   ap=[[0, p]] + list(ap.ap)[1:])


def tree_add20(nc, sc, src, out, M):
    """out (AP [128, M, 1] f32) = sum over the 20-wide inner axis of src
    (AP [128, M, 20] bf16, inner contiguous). sc = scratch tile dict.
    16+4 split keeps the wide levels 4B-aligned/bf16; the last levels run
    f32 so large partial sums don't get re-rounded to bf16."""
    t8, t4, t2, u2, t1, u1 = (sc[k] for k in ("t8", "t4", "t2", "u2", "t1", "u1"))
    nc.vector.tensor_tensor(out=t8[:, 0:M, :], in0=src[:, :, 0:8],
                            in1=src[:, :, 8:16], op=ALU.add)
    nc.vector.tensor_tensor(out=t4[:, 0:M, :], in0=t8[:, 0:M, 0:4],
                            in1=t8[:, 0:M, 4:8], op=ALU.add)
    nc.vector.tensor_tensor(out=t2[:, 0:M, :], in0=t4[:, 0:M, 0:2],
                            in1=t4[:, 0:M, 2:4], op=ALU.add)
    nc.vector.tensor_tensor(out=u2[:, 0:M, :], in0=src[:, :, 16:18],
                            in1=src[:, :, 18:20], op=ALU.add)
    nc.vector.tensor_tensor(out=t1[:, 0:M, :], in0=t2[:, 0:M, 0:1],
                            in1=t2[:, 0:M, 1:2], op=ALU.add)
    nc.vector.tensor_tensor(out=u1[:, 0:M, :], in0=u2[:, 0:M, 0:1],
                            in1=u2[:, 0:M, 1:2], op=ALU.add)
    nc.vector.tensor_tensor(out=out, in0=t1[:, 0:M, :], in1=u1[:, 0:M, :],
                            op=ALU.add)


def build_nc(sim=False):
    nc = bacc.Bacc("TRN2", target_bir_lowering=False, debug=False,
                   num_devices=1 if sim else NCORES)
    xw = nc.declare_dram_parameter("xw", [W, WIN, CP], BF16, isOutput=False)
    ohe = nc.declare_dram_parameter("ohe", [W, OWN, CP], BF16, isOutput=False)
    dw = nc.declare_dram_parameter("dw", [W + 2, EW], F32, isOutput=False)
    msk = nc.declare_dram_parameter("msk", [1, 2], F32, isOutput=False)
    etab = nc.declare_dram_parameter("etab", [1, 128], F32, isOutput=False)
    outp = nc.declare_dram_parameter("res", [1, 1], F32, isOutput=True)
    if DEBUG:
        dbgG = nc.declare_dram_parameter("dbgG", [1, 8], F32, isOutput=True)
        dbgC = nc.declare_dram_parameter("dbgC", [1, 128], F32, isOutput=True)
        dbgR = nc.declare_dram_parameter("dbgR", [1, 8], F32, isOutput=True)

    cnt_in = nc.dram_tensor("cnt_in", [1, 128], F32)
    cnt_out = nc.dram_tensor("cnt_out", [1, 128], F32, addr_space="Shared")
    fin_in = nc.dram_tensor("fin_in", [1, 8], F32)
    fin_out = nc.dram_tensor("fin_out", [1, 8], F32, addr_space="Shared")
    groups = [list(range(NCORES))]

    with tile.TileContext(nc) as tc, ExitStack() as ctx:
        keep = ctx.enter_context(tc.tile_pool(name="keep", bufs=1))
        big = ctx.enter_context(tc.tile_pool(name="big", bufs=1))
        tmp = ctx.enter_context(tc.tile_pool(name="tmp", bufs=1))
        med = ctx.enter_context(tc.tile_pool(name="med", bufs=1))
        psum = ctx.enter_context(tc.tile_pool(name="psum", bufs=1, space="PSUM"))
        dram = ctx.enter_context(tc.tile_pool(name="dram", bufs=1, space="DRAM"))

        # ---- persistent tiles ------------------------------------------
        lnS = [keep.tile([128, WIN], F32, tag=f"lnS{t}", name=f"lnS{t}")
               for t in range(4)]
        rS4 = [keep.tile([128, WIN], BF16, tag=f"rS{t}", name=f"rS{t}")
               for t in range(4)]
        E8 = [keep.tile([128, 8, EW], BF16, tag=f"E8{t}", name=f"E8{t}")
              for t in range(4)]
        klc4 = keep.tile([128, 4, EW], F32, tag="klc4")
        counts = keep.tile([128, KEPS], F32, tag="counts")
        stats = keep.tile([128, 5, 4], F32, tag="stats")
        nc.vector.memset(stats, 0.0)
        lsce4 = keep.tile([128, 4, OWN], BF16, tag="lsce4")
        M4 = keep.tile([128, 4, OWN], F32, tag="M4")
        nble4 = keep.tile([128, 4, OWN], BF16, tag="nble4")
        w4 = keep.tile([128, 4, OWN], BF16, tag="w4")
        mskb = keep.tile([128, 2], F32, tag="mskb")
        nc.sync.dma_start(out=mskb, in_=_bcast_part(msk[:]))
        etab_sb = keep.tile([1, 128], F32, tag="etab_sb")
        nc.sync.dma_start(out=etab_sb, in_=etab[:])
        ones = keep.tile([128, 1], F32, tag="ones")
        nc.vector.memset(ones, 1.0)
        cidx = keep.tile([128, 8], BF16, tag="cidx")
        for d in range(8):
            nc.vector.memset(cidx[:, d:d + 1], float(d))
        zrow = keep.tile([1, EW], F32, tag="zrow")
        nc.vector.memset(zrow, 0.0)
        zrowb = keep.tile([1, EW], BF16, tag="zrowb")
        nc.vector.memset(zrowb, 0.0)
        eps_dr = dram.tile([1, 1], F32, tag="eps_dr")

        # big pool: [128, WIN, CP] bf16 buffers, tags reused across phases
        def bigt(tag):
            return big.tile([128, WIN, CP], BF16, tag=tag, name=tag)

        # tree scratch (3-dir batched trees; S-tree uses M=WIN slice)
        MX = 3 * EW
        sc = {
            "t8": tmp.tile([128, MX, 8], BF16, tag="sc_t8", name="sc_t8"),
            "t4": tmp.tile([128, MX, 4], BF16, tag="sc_t4", name="sc_t4"),
            "t2": tmp.tile([128, MX, 2], BF16, tag="sc_t2", name="sc_t2"),
            "u2": tmp.tile([128, MX, 2], BF16, tag="sc_u2", name="sc_u2"),
            "t1": tmp.tile([128, MX, 1], F32, tag="sc_t1", name="sc_t1"),
            "u1": tmp.tile([128, MX, 1], F32, tag="sc_u1", name="sc_u1"),
        }
        prod = tmp.tile([128, MX, CP], BF16, tag="prod")

        # ================= Phase A: exp / S / lnS / 1/S / CE ============
        for t in range(4):
            p0 = t * 128
            xc = bigt("xc")
            nc.sync.dma_start(out=xc, in_=xw[p0:p0 + 128])
            ex = bigt("ex")
            nc.scalar.activation(out=ex, in_=xc, func=ACTF.Exp)
            S = med.tile([128, WIN, 1], F32, tag="S")
            tree_add20(nc, sc, ex[:, :, :], S[:], WIN)
            rSf = med.tile([128, WIN], F32, tag="lnL", name="rSf")
            nc.vector.reciprocal_approx_fast(out=rSf[:].unsqueeze(2), in_=S)
            nc.scalar.activation(out=rS4[t], in_=rSf, func=ACTF.Copy)
            # lnS (scalar engine); own rows also accumulate sum(lnS)
            nc.scalar.activation(out=lnS[t][:, 0:2], in_=S[:, 0:2, 0],
                                 func=ACTF.Ln)
            nc.scalar.activation(out=lnS[t][:, 2:2 + OWN],
                                 in_=S[:, 2:2 + OWN, 0], func=ACTF.Ln,
                                 accum_out=stats[:, 3, t:t + 1])
            nc.scalar.activation(out=lnS[t][:, 2 + OWN:WIN],
                                 in_=S[:, 2 + OWN:WIN, 0], func=ACTF.Ln)
            # CE data term: sum(onehot * x) over own rows
            oht = bigt("xl")
            nc.sync.dma_start(out=oht[:, 0:OWN, :], in_=ohe[p0:p0 + 128])
            junk = bigt("xr")
            nc.vector.scalar_tensor_tensor(
                out=junk[:, 0:OWN, :], in0=oht[:, 0:OWN, :], scalar=1.0,
                in1=xc[:, 2:2 + OWN, :], op0=ALU.mult, op1=ALU.mult,
                accum_out=stats[:, 4, t:t + 1])

        # ================= Phase B: 9 dots -> E8 / klc ==================
        for t in range(4):
            p0 = t * 128
            xc = bigt("xc")
            nc.sync.dma_start(out=xc, in_=xw[p0:p0 + 128])
            xL = bigt("xl")
            if t > 0:
                nc.sync.dma_start(out=xL, in_=xw[p0 - 1:p0 + 127])
            else:
                nc.sync.dma_start(out=xL[1:128], in_=xw[0:127])
                nc.sync.dma_start(out=xL[0:1], in_=xw[0:1])
            xR = bigt("xr")
            if t < 3:
                nc.sync.dma_start(out=xR, in_=xw[p0 + 1:p0 + 129])
            else:
                nc.sync.dma_start(out=xR[0:127], in_=xw[p0 + 1:p0 + 128])
                nc.sync.dma_start(out=xR[127:128], in_=xw[W - 1:W])
            lnL = med.tile([128, WIN], F32, tag="lnL")
            nc.sync.dma_start(out=lnL[1:128], in_=lnS[t][0:127])
            nc.sync.dma_start(out=lnL[0:1],
                              in_=(lnS[t - 1][127:128] if t > 0 else lnS[0][0:1]))
            lnR = med.tile([128, WIN], F32, tag="lnR")
            nc.sync.dma_start(out=lnR[0:127], in_=lnS[t][1:128])
            nc.sync.dma_start(out=lnR[127:128],
                              in_=(lnS[t + 1][0:1] if t < 3 else lnS[3][127:128]))
            # sm = exp(x) * (1/S); 1/S replicated across classes via DMA
            # (staged in prod, which the group mults overwrite afterwards)
            ex = bigt("ex")
            nc.scalar.activation(out=ex, in_=xc, func=ACTF.Exp)
            nc.scalar.activation(
                out=prod[:, 0:WIN, :],
                in_=rS4[t][:].unsqueeze(2).broadcast_to([128, WIN, CP]),
                func=ACTF.Copy)
            sm = bigt("sm")
            nc.vector.tensor_tensor(out=sm, in0=ex, in1=prod[:, 0:WIN, :],
                                    op=ALU.mult)

            D9 = med.tile([128, 9 * EW], F32, tag="D9")
            smc = sm[:, 1:1 + EW, :]
            lnX = {0: lnS[t], -1: lnL, 1: lnR}
            for (goff, dy) in GRP:
                xs = {0: xc, -1: xL, 1: xR}[dy]
                for j in range(3):
                    d = ORD[goff + j]
                    dx = 0 if d == 8 else DIRS[d][0]
                    nc.vector.tensor_tensor(
                        out=prod[:, j * EW:(j + 1) * EW, :], in0=smc,
                        in1=xs[:, 1 + dx:1 + dx + EW, :], op=ALU.mult)
                tree_add20(nc, sc, prod[:, 0:3 * EW, :],
                           D9[:, goff * EW:(goff + 3) * EW].unsqueeze(2),
                           3 * EW)

            # E8 = (A - D_d) + (lnS_d - lnS); A = D9 slab 2 (center)
            Abc2 = D9[:, 2 * EW:3 * EW].unsqueeze(1).broadcast_to([128, 2, EW])
            Abc6 = D9[:, 2 * EW:3 * EW].unsqueeze(1).broadcast_to([128, 6, EW])
            ad8 = med.tile([128, 8, EW], BF16, tag="b8c")
            nc.vector.scalar_tensor_tensor(
                out=ad8[:, 0:2, :],
                in0=D9[:, 0:2 * EW].rearrange("p (d e) -> p d e", e=EW),
                scalar=-1.0, in1=Abc2, op0=ALU.mult, op1=ALU.add)
            nc.vector.scalar_tensor_tensor(
                out=ad8[:, 2:8, :],
                in0=D9[:, 3 * EW:9 * EW].rearrange("p (d e) -> p d e", e=EW),
                scalar=-1.0, in1=Abc6, op0=ALU.mult, op1=ALU.add)
            Ld = med.tile([128, 8, EW], BF16, tag="b8d")
            lnSc = lnS[t][:, 1:1 + EW]
            for d in range(8):
                dx, dy = DIRS[d]
                nc.vector.tensor_tensor(
                    out=Ld[:, SLE[d], :],
                    in0=lnX[dy][:, 1 + dx:1 + dx + EW], in1=lnSc,
                    op=ALU.subtract)
            nc.vector.tensor_tensor(out=E8[t], in0=ad8, in1=Ld, op=ALU.add)

            # klc = E_(1,0) + E_(0,1) in f32 (threshold path needs the
            # precision; the bf16 E8 copy only feeds the lsce path)
            klc = klc4[:, t, :]
            e0f = med.tile([128, EW], F32, tag="e0f")
            nc.vector.scalar_tensor_tensor(
                out=e0f, in0=D9[:, 0:EW], scalar=-1.0,
                in1=D9[:, 2 * EW:3 * EW], op0=ALU.mult, op1=ALU.add)
            nc.vector.tensor_tensor(out=e0f, in0=e0f,
                                    in1=lnS[t][:, 2:2 + EW], op=ALU.add)
            e3f = med.tile([128, EW], F32, tag="e3f")
            nc.vector.scalar_tensor_tensor(
                out=e3f, in0=D9[:, 6 * EW:7 * EW], scalar=-1.0,
                in1=D9[:, 2 * EW:3 * EW], op0=ALU.mult, op1=ALU.add)
            nc.vector.tensor_tensor(out=e3f, in0=e3f,
                                    in1=lnR[:, 1:1 + EW], op=ALU.add)
            nc.vector.tensor_tensor(out=e0f, in0=e0f, in1=e3f, op=ALU.add)
            nc.vector.scalar_tensor_tensor(
                out=klc, in0=lnS[t][:, 1:1 + EW], scalar=-2.0,
                in1=e0f, op0=ALU.mult, op1=ALU.add)
            nc.vector.tensor_tensor(out=klc[:, 0:1], in0=klc[:, 0:1],
                                    in1=mskb[:, 0:1], op=ALU.mult)
            nc.vector.tensor_tensor(out=klc[:, EW - 1:EW], in0=klc[:, EW - 1:EW],
                                    in1=mskb[:, 1:2], op=ALU.mult)

        # ================= counts + AllReduce (overlapped) ==============
        # per-threshold 0/1 masks in bf16 (4x TS mode); TensorE reduces each
        # mask with accumulating column-sum matmuls into PSUM [128, KEPS]
        onesb = keep.tile([128, 1], BF16, tag="onesb")
        nc.vector.memset(onesb, 1.0)
        cpsum = psum.tile([128, KEPS], F32, tag="cpsum")
        NCH = 4 * OWN // 128
        for k in range(KEPS):
            mk = med.tile([128, 4, OWN], BF16, tag=f"msk{k % 2}",
                          name=f"msk{k % 2}")
            nc.vector.tensor_scalar(out=mk, in0=klc4[:, :, 1:1 + OWN],
                                    scalar1=EPS_LIST[k], scalar2=None,
                                    op0=ALU.is_gt)
            mf = mk[:].rearrange("p a b -> p (a b)")
            for c in range(NCH):
                nc.tensor.matmul(cpsum[:, k:k + 1],
                                 mf[:, c * 128:(c + 1) * 128], onesb,
                                 start=(c == 0), stop=(c == NCH - 1))
        nc.vector.tensor_copy(out=counts, in_=cpsum)
        cred = psum.tile([1, KEPS], F32, tag="cred")
        nc.tensor.matmul(cred, ones, counts, start=True, stop=True)
        cred_sb = keep.tile([1, KEPS], F32, tag="cred_sb")
        nc.vector.tensor_copy(out=cred_sb, in_=cred)
        nc.sync.dma_start(out=cnt_in[:, 0:KEPS], in_=cred_sb)
        if sim:
            nc.sync.dma_start(out=cnt_out[:, 0:KEPS], in_=cnt_in[:, 0:KEPS])
        else:
            nc.gpsimd.collective_compute(
                "AllReduce", ALU.add, replica_groups=groups,
                ins=[cnt_in[:, 0:KEPS]], outs=[cnt_out[:, 0:KEPS]])

        # ========== Phase D (eps-independent), overlaps the AllReduce ===
        for t in range(4):
            p0 = t * 128
            # dist9 stack from DRAM (slab order: DIRS + center at 8)
            D9d = med.tile([128, 9 * EW], F32, tag="D9")

            def dsl(di):
                return D9d[:, di * EW:di * EW + OWN]

            for di in range(9):
                dx, dy = (0, 0) if di == 8 else DIRS[di]
                nc.sync.dma_start(
                    out=dsl(di),
                    in_=dw[p0 + 1 + dy:p0 + 129 + dy][:, 1 + dx:1 + dx + OWN])
            f8b = med.tile([128, 4, EW], F32, tag="f8b")
            m4a, m2a = f8b[:, 0:4, 0:OWN], f8b[:, 0:2, 0:OWN]
            b8a = med.tile([128, 8, OWN], BF16, tag="b8a")  # pen/eqs/dsub
            b8b = med.tile([128, 8, OWN], BF16, tag="b8b")  # kp/dexp
            b4 = med.tile([128, 4, OWN], BF16, tag="b4")
            b2 = med.tile([128, 2, OWN], BF16, tag="b2")
            d8v = D9d[:, 0:8 * EW].rearrange(
                "p (d e) -> p d e", e=EW)[:, :, 0:OWN]
            nc.vector.tensor_tensor(out=m4a, in0=d8v[:, 0:4, :],
                                    in1=d8v[:, 4:8, :], op=ALU.min)
            nc.vector.tensor_tensor(out=m2a, in0=m4a[:, 0:2, :],
                                    in1=m4a[:, 2:4, :], op=ALU.min)
            min8 = med.tile([128, OWN], F32, tag="min8")
            nc.vector.tensor_tensor(out=min8, in0=m2a[:, 0, :],
                                    in1=m2a[:, 1, :], op=ALU.min)
            nc.vector.tensor_tensor(out=nble4[:, t, :], in0=min8,
                                    in1=dsl(8), op=ALU.is_le)
            nc.vector.tensor_scalar(out=w4[:, t, :], in0=dsl(8),
                                    scalar1=20.0, scalar2=0.05,
                                    op0=ALU.min, op1=ALU.mult)
            # first-argmin index among the 8 dirs (small ints: exact bf16)
            nc.vector.tensor_tensor(
                out=b8a, in0=d8v,
                in1=min8[:].unsqueeze(1).broadcast_to([128, 8, OWN]),
                op=ALU.is_equal)
            nc.vector.scalar_tensor_tensor(
                out=b8a, in0=b8a, scalar=-100.0, op0=ALU.mult,
                in1=cidx[:].unsqueeze(2).broadcast_to([128, 8, OWN]),
                op1=ALU.add)
            nc.vector.tensor_tensor(out=b4, in0=b8a[:, 0:4, :],
                                    in1=b8a[:, 4:8, :], op=ALU.min)
            nc.vector.tensor_tensor(out=b2, in0=b4[:, 0:2, :],
                                    in1=b4[:, 2:4, :], op=ALU.min)
            dgt = med.tile([128, OWN], BF16, tag="dgt")
            nc.vector.tensor_tensor(out=dgt, in0=b2[:, 0, :],
                                    in1=b2[:, 1, :], op=ALU.min)
            nc.vector.tensor_scalar(out=dgt, in0=dgt, scalar1=100.0,
                                    scalar2=None, op0=ALU.add)

            # K8 stack: K_d = E_{NEG[d]} shifted by d
            K8 = med.tile([128, 8, OWN], BF16, tag="K8")
            for di in range(8):
                dx, dy = DIRS[di]
                sl = SLE[NEG[di]]
                if dy == 0:
                    nc.scalar.activation(
                        out=K8[:, di, :],
                        in_=E8[t][:, sl, 1 + dx:1 + dx + OWN], func=ACTF.Copy)
                elif dy == -1:
                    nc.sync.dma_start(
                        out=K8[1:128, di, :],
                        in_=E8[t][0:127, sl, 1 + dx:1 + dx + OWN])
                    if t > 0:
                        nc.sync.dma_start(
                            out=K8[0:1, di, :],
                            in_=E8[t - 1][127:128, sl, 1 + dx:1 + dx + OWN])
                    elif dx != 0:
                        gsl = SLE[DIRS.index((-dx, 0))]
                        nc.sync.dma_start(
                            out=K8[0:1, di, :],
                            in_=E8[0][0:1, gsl, 1 + dx:1 + dx + OWN])
                    else:
                        nc.sync.dma_start(out=K8[0:1, di, :],
                                          in_=zrowb[:, 0:OWN])
                else:
                    nc.sync.dma_start(
                        out=K8[0:127, di, :],
                        in_=E8[t][1:128, sl, 1 + dx:1 + dx + OWN])
                    if t < 3:
                        nc.sync.dma_start(
                            out=K8[127:128, di, :],
                            in_=E8[t + 1][0:1, sl, 1 + dx:1 + dx + OWN])
                    elif dx != 0:
                        gsl = SLE[DIRS.index((-dx, 0))]
                        nc.sync.dma_start(
                            out=K8[127:128, di, :],
                            in_=E8[3][127:128, gsl, 1 + dx:1 + dx + OWN])
                    else:
                        nc.sync.dma_start(out=K8[127:128, di, :],
                                          in_=zrowb[:, 0:OWN])

            # ksel = K8[dgt] via equality select
            nc.vector.tensor_tensor(
                out=b8a, in0=cidx[:].unsqueeze(2).broadcast_to([128, 8, OWN]),
                in1=dgt[:].unsqueeze(1).broadcast_to([128, 8, OWN]),
                op=ALU.is_equal)
            nc.vector.tensor_tensor(out=b8b, in0=b8a, in1=K8, op=ALU.mult)
            nc.vector.tensor_tensor(out=b4, in0=b8b[:, 0:4, :],
                                    in1=b8b[:, 4:8, :], op=ALU.add)
            nc.vector.tensor_tensor(out=b2, in0=b4[:, 0:2, :],
                                    in1=b4[:, 2:4, :], op=ALU.add)
            ksel = med.tile([128, OWN], BF16, tag="ksel")
            nc.vector.tensor_tensor(out=ksel, in0=b2[:, 0, :],
                                    in1=b2[:, 1, :], op=ALU.add)

            # LSE + plain sum over the 8 K maps
            nc.vector.tensor_tensor(out=b4, in0=K8[:, 0:4, :],
                                    in1=K8[:, 4:8, :], op=ALU.max)
            nc.vector.tensor_tensor(out=b2, in0=b4[:, 0:2, :],
                                    in1=b4[:, 2:4, :], op=ALU.max)
            m8 = med.tile([128, OWN], BF16, tag="m8")
            nc.vector.tensor_tensor(out=m8, in0=b2[:, 0, :],
                                    in1=b2[:, 1, :], op=ALU.max)
            nc.vector.tensor_tensor(
                out=b8a, in0=K8,
                in1=m8[:].unsqueeze(1).broadcast_to([128, 8, OWN]),
                op=ALU.subtract)
            nc.scalar.activation(out=b8b, in_=b8a, func=ACTF.Exp)
            nc.vector.tensor_tensor(out=b4, in0=b8b[:, 0:4, :],
                                    in1=b8b[:, 4:8, :], op=ALU.add)
            nc.vector.tensor_tensor(out=b2, in0=b4[:, 0:2, :],
                                    in1=b4[:, 2:4, :], op=ALU.add)
            esum = med.tile([128, OWN], BF16, tag="esum")
            nc.vector.tensor_tensor(out=esum, in0=b2[:, 0, :],
                                    in1=b2[:, 1, :], op=ALU.add)
            lnE = med.tile([128, OWN], BF16, tag="lnE")
            nc.scalar.activation(out=lnE, in_=esum, func=ACTF.Ln)
            lse = med.tile([128, OWN], BF16, tag="lse")
            nc.vector.tensor_tensor(out=lse, in0=m8, in1=lnE, op=ALU.add)
            nc.vector.tensor_tensor(out=b4, in0=K8[:, 0:4, :],
                                    in1=K8[:, 4:8, :], op=ALU.add)
            nc.vector.tensor_tensor(out=b2, in0=b4[:, 0:2, :],
                                    in1=b4[:, 2:4, :], op=ALU.add)
            s8 = med.tile([128, OWN], BF16, tag="s8")
            nc.vector.tensor_tensor(out=s8, in0=b2[:, 0, :],
                                    in1=b2[:, 1, :], op=ALU.add)
            # lsce = SSUM*lse - LB_NEG*s8 - (LB_POS-LB_NEG)*ksel
            a1 = med.tile([128, OWN], BF16, tag="a1")
            nc.vector.tensor_scalar(out=a1, in0=s8, scalar1=-LB_NEG,
                                    scalar2=None, op0=ALU.mult)
            b1 = med.tile([128, OWN], BF16, tag="b1")
            nc.vector.scalar_tensor_tensor(out=b1, in0=lse, scalar=SSUM,
                                           op0=ALU.mult, in1=a1, op1=ALU.add)
            nc.vector.scalar_tensor_tensor(out=lsce4[:, t, :], in0=ksel,
                                           scalar=-(LB_POS - LB_NEG),
                                           op0=ALU.mult, in1=b1, op1=ALU.add)

            # dilation M = 3x3 max of klc
            kC = klc4[:, t, :]
            kL = med.tile([128, EW], F32, tag="kL")
            if t > 0:
                nc.sync.dma_start(out=kL[0:1], in_=klc4[127:128, t - 1, :])
            else:
                nc.vector.memset(kL[0:1], 0.0)
            nc.sync.dma_start(out=kL[1:128], in_=klc4[0:127, t, :])
            kR = med.tile([128, EW], F32, tag="kR")
            if t < 3:
                nc.sync.dma_start(out=kR[127:128], in_=klc4[0:1, t + 1, :])
            else:
                nc.sync.dma_start(out=kR[127:128], in_=zrow)
            nc.sync.dma_start(out=kR[0:127], in_=klc4[1:128, t, :])
            M = M4[:, t, :]
            nc.vector.tensor_tensor(out=M, in0=kL[:, 0:OWN],
                                    in1=kL[:, 1:1 + OWN], op=ALU.max)
            nc.vector.tensor_tensor(out=M, in0=M, in1=kL[:, 2:2 + OWN],
                                    op=ALU.max)
            for src in (kC, kR):
                for rs in range(3):
                    nc.vector.tensor_tensor(out=M, in0=M,
                                            in1=src[:, rs:rs + OWN],
                                            op=ALU.max)

        # ================= eps + masked sums ============================
        tot = keep.tile([1, KEPS], F32, tag="tot")
        nc.sync.dma_start(out=tot, in_=cnt_out[:, 0:KEPS])
        maskT = keep.tile([1, KEPS], F32, tag="maskT")
        nc.vector.tensor_scalar(out=maskT, in0=tot, scalar1=MAX_N, scalar2=None,
                                op0=ALU.is_le)
        penal = keep.tile([1, KEPS], F32, tag="penal")
        nc.vector.tensor_scalar(out=penal, in0=maskT, scalar1=-1e30,
                                scalar2=1e30, op0=ALU.mult, op1=ALU.add)
        maskedT = keep.tile([1, KEPS], F32, tag="maskedT")
        nc.vector.tensor_tensor(out=maskedT, in0=etab_sb[:, 0:KEPS], in1=penal,
                                op=ALU.add)
        eps1 = keep.tile([1, 1], F32, tag="eps1")
        nc.vector.tensor_reduce(out=eps1, in_=maskedT, axis=AX.X, op=ALU.min)
        nc.sync.dma_start(out=eps_dr[:], in_=eps1)
        epsb = keep.tile([128, 1], F32, tag="epsb")
        nc.sync.dma_start(out=epsb, in_=_bcast_part(eps_dr[:]))

        for t in range(4):
            pb = med.tile([128, OWN], BF16, tag="pb")
            nc.vector.tensor_scalar(out=pb, in0=M4[:, t, :], scalar1=epsb,
                                    scalar2=None, op0=ALU.is_gt)
            vm = med.tile([128, OWN], BF16, tag="vm")
            nc.vector.tensor_tensor(out=vm, in0=pb, in1=nble4[:, t, :],
                                    op=ALU.mult)
            junkD = med.tile([128, OWN], BF16, tag="junkD")
            nc.vector.scalar_tensor_tensor(out=junkD, in0=lsce4[:, t, :],
                                           scalar=1.0, op0=ALU.mult, in1=vm,
                                           op1=ALU.mult,
                                           accum_out=stats[:, 0, t:t + 1])
            nc.vector.scalar_tensor_tensor(out=junkD, in0=w4[:, t, :],
                                           scalar=1.0, op0=ALU.mult, in1=vm,
                                           op1=ALU.mult,
                                           accum_out=stats[:, 1, t:t + 1])
            nc.vector.tensor_scalar(out=junkD, in0=pb, scalar1=1.0, scalar2=0.0,
                                    op0=ALU.mult, op1=ALU.add,
                                    accum_out=stats[:, 2, t:t + 1])

        # ================= final reduce + AllReduce + output ============
        red5 = keep.tile([128, 5], F32, tag="red5")
        nc.vector.tensor_reduce(out=red5, in_=stats, axis=AX.X, op=ALU.add)
        redr = psum.tile([1, 5], F32, tag="redr")
        nc.tensor.matmul(redr, ones, red5, start=True, stop=True)
        redr_sb = keep.tile([1, 5], F32, tag="redr_sb")
        nc.vector.tensor_copy(out=redr_sb, in_=redr)
        nc.sync.dma_start(out=fin_in[:, 0:5], in_=redr_sb)
        if sim:
            nc.sync.dma_start(out=fin_out[:, 0:5], in_=fin_in[:, 0:5])
        else:
            nc.gpsimd.collective_compute(
                "AllReduce", ALU.add, replica_groups=groups,
                ins=[fin_in[:, 0:5]], outs=[fin_out[:, 0:5]])
        G = keep.tile([1, 5], F32, tag="G")
        nc.sync.dma_start(out=G, in_=fin_out[:, 0:5])
        if DEBUG:
            nc.sync.dma_start(out=dbgG[:, 0:5], in_=G)
            nc.sync.dma_start(out=dbgG[:, 5:6], in_=eps1)
            nc.sync.dma_start(out=dbgC[:, 0:KEPS], in_=tot)
            nc.sync.dma_start(out=dbgR[:, 0:5], in_=redr_sb)
        gate = keep.tile([1, 1], F32, tag="gate")
        nc.vector.tensor_scalar(out=gate, in0=G[:, 2:3], scalar1=1.0,
                                scalar2=None, op0=ALU.is_gt)
        bl = keep.tile([1, 1], F32, tag="bl")
        nc.vector.tensor_tensor(out=bl, in0=G[:, 0:1], in1=G[:, 1:2],
                                op=ALU.mult)
        nc.vector.tensor_tensor(out=bl, in0=bl, in1=gate, op=ALU.mult)
        tl = keep.tile([1, 1], F32, tag="tl")
        nc.vector.tensor_tensor(out=tl, in0=G[:, 3:4], in1=G[:, 4:5],
                                op=ALU.subtract)
        res = keep.tile([1, 1], F32, tag="res")
        nc.vector.scalar_tensor_tensor(out=res, in0=bl, scalar=0.1,
                                       in1=tl, op0=ALU.mult, op1=ALU.add)
        nc.sync.dma_start(out=outp[:], in_=res)

    nc.compile()
    return nc


_NC = None


def _get_nc():
    global _NC
    if _NC is None:
        _NC = build_nc()
    return _NC


def kernel_in_maps(slices, dist_maps, targets):
    slices = np.asarray(slices, np.float32)
    dist_maps = np.asarray(dist_maps, np.float32)
    targets = np.asarray(targets)
    etab = np.zeros((1, 128), np.float32)
    etab[0, :KEPS] = EPS_LIST
    etab[0, KEPS:] = EPS_LIST[-1]
    in_maps = []
    for core in range(NCORES):
        b, hf = core // 2, core % 2
        r0 = hf * OWN
        rows = np.clip(np.arange(r0 - 2, r0 + OWN + 2), 0, H - 1)
        xwin = np.full((W, WIN, CP), XPAD, np.float32)
        xwin[:, :, :C] = np.transpose(slices[b][:, rows, :], (2, 1, 0))
        xwv = xwin.astype(bfloat16)
        tgt = targets[b, 0, r0:r0 + OWN]                     # [OWN, W]
        oh = np.zeros((W, OWN, CP), np.float32)
        ww, rr = np.meshgrid(np.arange(W), np.arange(OWN), indexing='ij')
        cc = np.clip(tgt.T, 0, C - 1)                        # [W, OWN]
        oh[ww, rr, cc] = np.where(tgt.T == 255, 0.0, 1.0)
        ohv = oh.astype(bfloat16)
        ridx = np.arange(r0 - 1, r0 + OWN + 1)
        inb = ((ridx >= 0) & (ridx < H))[:, None]
        dwin = np.where(inb, dist_maps[b, 0][np.clip(ridx, 0, H - 1)],
                        np.float32(1e5))                      # [EW, W]
        dwin = np.pad(dwin, ((0, 0), (1, 1)),
                      constant_values=np.float32(1e5))        # [EW, W+2]
        dwv = np.ascontiguousarray(dwin.T)                    # [W+2, EW]
        mskv = np.array([[1.0 if r0 > 0 else 0.0,
                          1.0 if r0 + OWN < H else 0.0]], np.float32)
        in_maps.append({"xw": xwv, "ohe": ohv, "dw": dwv, "msk": mskv,
                        "etab": etab})
    return in_maps


def kernel(slices, dist_maps, targets):
    in_maps = kernel_in_maps(slices, dist_maps, targets)
    nc = _get_nc()
    res = run_bass_kernel_spmd(nc, in_maps, list(range(NCORES)))
    out = np.asarray(res.results[0]["res"], np.float32)
    return out.reshape(())
